# revision 39
# baseline (speedup 1.0000x reference)
"""Trainium2 Bass kernel for nn_AttnAdapter (GQA attention + RoPE + ClearSight
VAF region scaling + causal softmax), tensor-parallel over heads on 8 cores.

Sharding (Megatron-style TP): core c owns q-heads 4c..4c+3 and kv-head c.
hidden_states^T is replicated to every core (host-side); Wq/Wk/Wv are
column-sharded. o_proj is row-parallel: each core contracts only its own 4
heads over all 4096 output dims, and per seq-quarter a ReduceScatter(add)
sums the partials across cores while scattering output dims, so core c ends
up with its own 512 output columns. The quarter ReduceScatters overlap the
next quarter's attention compute. Output is produced as 4 contiguous
[512, 512] bf16 quarters, assembled + upcast to f32 on the host.

Timing: `_build(reps=R)` statically unrolls the whole body R times (distinct
collective instructions per repetition — collectives inside a hardware loop
desync NRT). HW exec time per invocation = (marginal wall of the R-NEFF -
marginal wall of the 1-NEFF) / (R-1), each marginal measured from pipelined
async execute batches so launch overhead cancels.
"""

import time

import numpy as np
import ml_dtypes

import concourse.bass as bass
import concourse.mybir as mybir
import concourse.tile as tile
from concourse import bacc
from concourse.bass import ts

N_CORES = 8
P = 128
S = 2048
H = 4096
HD = 128
HQ = 4              # q heads per core
JW = 512            # qs super-tile width
NJ = S // JW        # 4
NT = S // P         # 16
KH = H // P         # 32 contraction tiles for projections
SYS, IMG = 35, 576
B = SYS + IMG       # 611: first query row with VAF scaling
ENH, SUP = 2.0, 0.1
FT = -(-B // P)     # 5: ks-tiles with non-unit VAF factor
SCALING = HD ** -0.5

F32 = mybir.dt.float32
MM_DT = mybir.dt.bfloat16
NP_DT = ml_dtypes.bfloat16
RS_DT = MM_DT       # o_proj partials + ReduceScatter in bf16


def _build(reps=1, with_cc=True):
    """with_cc=False drops the ReduceScatters (timing diagnostic only)."""
    nc = bacc.Bacc("TRN2", target_bir_lowering=False, debug=False,
                   num_devices=N_CORES)

    hsT = nc.dram_tensor("hsT", [KH, P, S], MM_DT, kind="ExternalInput")
    wq = nc.dram_tensor("wq", [H, HQ * HD], MM_DT, kind="ExternalInput")
    wk = nc.dram_tensor("wk", [H, HD], MM_DT, kind="ExternalInput")
    wv = nc.dram_tensor("wv", [H, HD], MM_DT, kind="ExternalInput")
    wo = nc.dram_tensor("wo", [HQ * HD, H], MM_DT, kind="ExternalInput")
    cosT = nc.dram_tensor("cosT", [P, S], MM_DT, kind="ExternalInput")
    sinT = nc.dram_tensor("sinT", [P, S], MM_DT, kind="ExternalInput")
    rotT = nc.dram_tensor("rotT", [P, P], MM_DT, kind="ExternalInput")
    triT = nc.dram_tensor("triT", [P, P], MM_DT, kind="ExternalInput")
    fmask = nc.dram_tensor("fmask", [P, FT * P], MM_DT, kind="ExternalInput")
    idn = nc.dram_tensor("idn", [P, P], MM_DT, kind="ExternalInput")
    ones_col = nc.dram_tensor("ones_col", [P, 1], MM_DT, kind="ExternalInput")
    fvecT = nc.dram_tensor("fvecT", [P, FT], F32, kind="ExternalInput")
    outQ = [nc.dram_tensor(f"outQ{j}", [JW, JW], RS_DT, kind="ExternalOutput")
            for j in range(NJ)]

    with tile.TileContext(nc) as tc:
        with (
            tc.tile_pool(name="dram", bufs=1, space="DRAM") as dpool,
            tc.tile_pool(name="consts", bufs=1) as cpool,
            tc.tile_pool(name="qkv", bufs=1) as qkv_pool,
            tc.tile_pool(name="pjw", bufs=1) as pjw,
            tc.tile_pool(name="hs_pool", bufs=8) as hs_pool,
            tc.tile_pool(name="rp_tmp", bufs=4) as rp_tmp,
            tc.tile_pool(name="strip", bufs=4) as strip_pool,
            tc.tile_pool(name="norm", bufs=3) as norm_pool,
            tc.tile_pool(name="fin_pool", bufs=4) as fin_pool,
        ):
            partial = [dpool.tile([H, JW], RS_DT, name=f"partial{j}")
                       for j in range(NJ)]
            outRS = [dpool.tile([JW, JW], RS_DT, name=f"outRS{j}")
                     for j in range(NJ)]

            rot_sb = cpool.tile([P, P], MM_DT, name="rot_sb")
            tri_sb = cpool.tile([P, P], MM_DT, name="tri_sb")
            fm_sb = cpool.tile([P, FT * P], MM_DT, name="fm_sb")
            idn_sb = cpool.tile([P, P], MM_DT, name="idn_sb")
            ones_sb = cpool.tile([P, 1], MM_DT, name="ones_sb")
            fv_sb = cpool.tile([P, FT], F32, name="fv_sb")
            nc.sync.dma_start(rot_sb[:], rotT[:])
            nc.sync.dma_start(tri_sb[:], triT[:])
            nc.sync.dma_start(fm_sb[:], fmask[:])
            nc.sync.dma_start(idn_sb[:], idn[:])
            nc.sync.dma_start(ones_sb[:], ones_col[:])
            nc.sync.dma_start(fv_sb[:], fvecT[:])

            qT = qkv_pool.tile([P, HQ, S], MM_DT, name="qT")
            kT = qkv_pool.tile([P, S], MM_DT, name="kT")
            v_sb = qkv_pool.tile([P, NT, HD], MM_DT, name="v_sb")
            kTs = qkv_pool.tile([P, FT * P], MM_DT, name="kTs")
            oT_sb = qkv_pool.tile([P, HQ, S], MM_DT, name="oT_sb")
            wo_sb = qkv_pool.tile([P, HQ, H], MM_DT, name="wo_sb")
            cos_sb = pjw.tile([P, S], MM_DT, name="cos_sb")
            sin_sb = pjw.tile([P, S], MM_DT, name="sin_sb")
            vT = pjw.tile([P, S], MM_DT, name="vT")
            wq_sb = pjw.tile([P, KH, HQ * HD], MM_DT, name="wq_sb")
            wk_sb = pjw.tile([P, KH, HD], MM_DT, name="wk_sb")
            wv_sb = pjw.tile([P, KH, HD], MM_DT, name="wv_sb")
            wo3 = wo.rearrange("(k p) m -> p k m", p=P)
            wq3 = wq.rearrange("(k p) m -> p k m", p=P)
            wk3 = wk.rearrange("(k p) m -> p k m", p=P)
            wv3 = wv.rearrange("(k p) m -> p k m", p=P)
            WC = 4  # k-tiles per weight-load chunk

            for R in range(reps):
                # Each n-slice: projections + RoPE + V-transposes for seq
                # slice n, immediately followed by attention + o_proj + RS
                # for quarter J=n (which only needs slices <= n). This puts
                # the first ReduceScatter ~20% into the invocation so all
                # but the last one hide under compute.
                targets = [qT[:, h, :] for h in range(HQ)] + [kT[:]]
                for n in range(NJ):
                  with (
                      tc.tile_pool(name=f"pj_psum{R}_{n}", bufs=1,
                                   space="PSUM") as pj_psum,
                      tc.tile_pool(name=f"rp_psum{R}_{n}", bufs=2,
                                   space="PSUM") as rp_psum,
                  ):
                    ps_q = [pj_psum.tile([P, JW], F32, tag=f"psq{h}",
                                         name=f"psq{h}_{n}_{R}")
                            for h in range(HQ)]
                    ps_k = pj_psum.tile([P, JW], F32, tag="psk",
                                        name=f"psk_{n}_{R}")
                    ps_v = pj_psum.tile([P, JW], F32, tag="psv",
                                        name=f"psv_{n}_{R}")
                    for k in range(KH):
                        if n == 0 and k % WC == 0:
                            # stream weight chunks just ahead of use
                            nc.sync.dma_start(wq_sb[:, k:k + WC, :],
                                              wq3[:, k:k + WC, :])
                            nc.sync.dma_start(wk_sb[:, k:k + WC, :],
                                              wk3[:, k:k + WC, :])
                            nc.sync.dma_start(wv_sb[:, k:k + WC, :],
                                              wv3[:, k:k + WC, :])
                            # big loads not needed until later: spread them
                            # out so no hst tile queues behind a MB-scale DMA
                            i = k // WC - 1
                            if 0 <= i < HQ:
                                nc.sync.dma_start(wo_sb[:, i, :],
                                                  wo3[:, i, :])
                            elif i == HQ:
                                nc.sync.dma_start(cos_sb[:], cosT[:])
                            elif i == HQ + 1:
                                nc.sync.dma_start(sin_sb[:], sinT[:])
                        hst = hs_pool.tile([P, JW], MM_DT, tag="hs",
                                           name=f"hs_{n}_{k}_{R}")
                        nc.sync.dma_start(hst[:], hsT[k, :, ts(n, JW)])
                        st, sp = (k == 0), (k == KH - 1)
                        for h in range(HQ):
                            nc.tensor.matmul(ps_q[h][:],
                                             wq_sb[:, k, ts(h, HD)],
                                             hst[:], start=st, stop=sp)
                        nc.tensor.matmul(ps_k[:], wk_sb[:, k, :],
                                         hst[:], start=st, stop=sp)
                        nc.tensor.matmul(ps_v[:], wv_sb[:, k, :],
                                         hst[:], start=st, stop=sp)
                    for h in range(HQ):
                        nc.vector.tensor_copy(qT[:, h, ts(n, JW)],
                                              ps_q[h][:])
                    nc.vector.tensor_copy(kT[:, ts(n, JW)], ps_k[:])
                    nc.vector.tensor_copy(vT[:, ts(n, JW)], ps_v[:])
                    # RoPE for this n-slice: x <- x*cos + (Rot@x)*sin
                    for i, tgt in enumerate(targets):
                        rps = rp_psum.tile([P, JW], F32, tag="rp",
                                           name=f"rp_{i}_{n}_{R}")
                        nc.tensor.matmul(rps[:], rot_sb[:],
                                         tgt[:, ts(n, JW)],
                                         start=True, stop=True)
                        tmp = rp_tmp.tile([P, JW], MM_DT, tag="rt",
                                          name=f"rt_{i}_{n}_{R}")
                        nc.vector.tensor_mul(tmp[:], rps[:],
                                             sin_sb[:, ts(n, JW)])
                        nc.vector.tensor_mul(tgt[:, ts(n, JW)],
                                             tgt[:, ts(n, JW)],
                                             cos_sb[:, ts(n, JW)])
                        nc.vector.tensor_add(tgt[:, ts(n, JW)],
                                             tgt[:, ts(n, JW)], tmp[:])

                  with tc.tile_pool(name=f"tr_psum{R}_{n}", bufs=2,
                                    space="PSUM") as tr_psum:
                    for t in range(4 * n, 4 * n + 4):
                        tp = tr_psum.tile([P, P], MM_DT, tag="tr",
                                          name=f"tr_{t}_{R}")
                        nc.tensor.transpose(tp[:], vT[:, ts(t, P)],
                                            idn_sb[:])
                        nc.vector.tensor_copy(v_sb[:, t, :], tp[:])
                  if n == 1:
                    # VAF-scaled keys (first 640 ks) — needs slices 0 and 1
                    nc.vector.tensor_mul(kTs[:], kT[:, 0:FT * P], fm_sb[:])

                  with (
                      tc.tile_pool(name=f"sc_psum{R}_{n}", bufs=3,
                                   space="PSUM") as sc_psum,
                      tc.tile_pool(name=f"ao_psum{R}_{n}", bufs=2,
                                   space="PSUM") as ao_psum,
                      tc.tile_pool(name=f"dn_psum{R}_{n}", bufs=1,
                                   space="PSUM") as dn_psum,
                      tc.tile_pool(name=f"op_psum{R}_{n}", bufs=2,
                                   space="PSUM") as op_psum,
                  ):
                    J = n
                    qlo, qhi = J * JW, (J + 1) * JW
                    tmax = qhi // P - 1
                    for h in range(HQ):
                        otp = ao_psum.tile([P, JW], F32, tag="ot",
                                           name=f"ot_{h}_{J}_{R}")
                        dnp = dn_psum.tile([1, JW], F32, tag="dn",
                                           name=f"dn_{h}_{J}_{R}")
                        for t in range(tmax + 1):
                            o = max(0, t * P - qlo)
                            scp = sc_psum.tile([P, JW], F32, tag="sc",
                                               name=f"sc_{h}_{J}_{t}_{R}")
                            q_ap = qT[:, h, :]
                            needs_vaf = (t * P < B) and (qhi > B)
                            split = max(o, B - qlo) if needs_vaf else JW
                            if needs_vaf and split == o:
                                # entire strip in the VAF region
                                nc.tensor.matmul(
                                    scp[:, o:JW], kTs[:, ts(t, P)],
                                    q_ap[:, qlo + o:qhi],
                                    start=True, stop=True)
                            else:
                                nc.tensor.matmul(
                                    scp[:, o:JW], kT[:, ts(t, P)],
                                    q_ap[:, qlo + o:qhi],
                                    start=True, stop=True)
                                if needs_vaf and split < JW:
                                    # straddling strip: scale the qs >= B
                                    # columns by the per-ks VAF factor
                                    nc.vector.tensor_scalar_mul(
                                        scp[:, split:JW], scp[:, split:JW],
                                        fv_sb[:, t:t + 1])
                            strip = strip_pool.tile([P, JW], MM_DT, tag="st",
                                                    name=f"st_{h}_{J}_{t}_{R}")
                            nc.scalar.activation(
                                strip[:, o:JW], scp[:, o:JW],
                                mybir.ActivationFunctionType.Exp)
                            if t * P >= qlo:  # diagonal block
                                nc.vector.tensor_mul(strip[:, o:o + P],
                                                     strip[:, o:o + P],
                                                     tri_sb[:])
                            st, sp = (t == 0), (t == tmax)
                            nc.tensor.matmul(otp[:, o:JW],
                                             v_sb[:, t, :],
                                             strip[:, o:JW],
                                             start=st, stop=sp)
                            nc.tensor.matmul(dnp[:, o:JW], ones_sb[:],
                                             strip[:, o:JW],
                                             start=st, stop=sp)
                        recip = norm_pool.tile([1, JW], F32, tag="rc",
                                               name=f"rc_{h}_{J}_{R}")
                        nc.vector.reciprocal(recip[:], dnp[:])
                        bc = norm_pool.tile([P, JW], F32, tag="bc",
                                            name=f"bc_{h}_{J}_{R}")
                        nc.gpsimd.partition_broadcast(bc[:], recip[:])
                        nc.vector.tensor_mul(oT_sb[:, h, ts(J, JW)],
                                             otp[:], bc[:])
                    # row-parallel o_proj partial over own 4 heads only
                    for od in range(KH):
                        pp = op_psum.tile([P, JW], F32, tag="op",
                                          name=f"op_{od}_{J}_{R}")
                        for k in range(HQ):
                            nc.tensor.matmul(
                                pp[:], wo_sb[:, k, ts(od, P)],
                                oT_sb[:, k, ts(J, JW)],
                                start=(k == 0), stop=(k == HQ - 1))
                        fin = fin_pool.tile([P, JW], RS_DT, tag="fin",
                                            name=f"fin_{od}_{J}_{R}")
                        nc.vector.tensor_copy(fin[:], pp[:])
                        nc.sync.dma_start(partial[J][ts(od, P), :], fin[:])
                    # sum partials across cores; core c receives its own
                    # 512 output columns of this seq-quarter, transposed
                    if with_cc:
                        nc.gpsimd.collective_compute(
                            "ReduceScatter", mybir.AluOpType.add,
                            replica_groups=[list(range(N_CORES))],
                            ins=[partial[J].opt()],
                            outs=[outRS[J].opt()],
                        )
                        # collectives cannot write IO tensors directly
                        nc.sync.dma_start(outQ[J][:], outRS[J].opt())

    nc.compile()
    return nc


def _host_inputs(hidden_states, cos, sin, Wq, Wk, Wv, Wo):
    hs2d = np.asarray(hidden_states, dtype=np.float32).reshape(S, H)
    hsT_np = np.ascontiguousarray(hs2d.T).astype(NP_DT).reshape(KH, P, S)
    cosT_np = np.ascontiguousarray(
        np.asarray(cos, np.float32).reshape(S, HD).T).astype(NP_DT)
    sinT_np = np.ascontiguousarray(
        np.asarray(sin, np.float32).reshape(S, HD).T).astype(NP_DT)

    rot = np.zeros((HD, HD), np.float32)
    for i in range(HD // 2):
        rot[i, i + HD // 2] = -1.0
        rot[i + HD // 2, i] = 1.0
    rotT_np = np.ascontiguousarray(rot.T).astype(NP_DT)

    triT_np = np.triu(np.ones((P, P), np.float32)).astype(NP_DT)
    f = np.ones(FT * P, np.float32)
    f[:SYS] = SUP
    f[SYS:B] = ENH
    fmask_np = np.ascontiguousarray(
        np.broadcast_to(f, (P, FT * P))).astype(NP_DT)
    idn_np = np.eye(P, dtype=np.float32).astype(NP_DT)
    ones_np = np.ones((P, 1), np.float32).astype(NP_DT)
    fvecT_np = np.ascontiguousarray(f.reshape(FT, P).T)

    Wq = (np.asarray(Wq, np.float32) * np.float32(SCALING)).astype(NP_DT)
    Wk = np.asarray(Wk, np.float32).astype(NP_DT)
    Wv = np.asarray(Wv, np.float32).astype(NP_DT)
    Wo = np.asarray(Wo, np.float32).astype(NP_DT)

    in_maps = []
    for c in range(N_CORES):
        in_maps.append({
            "hsT": hsT_np,
            "wq": np.ascontiguousarray(Wq[:, c * HQ * HD:(c + 1) * HQ * HD]),
            "wk": np.ascontiguousarray(Wk[:, c * HD:(c + 1) * HD]),
            "wv": np.ascontiguousarray(Wv[:, c * HD:(c + 1) * HD]),
            "wo": np.ascontiguousarray(Wo[c * HQ * HD:(c + 1) * HQ * HD, :]),
            "cosT": cosT_np, "sinT": sinT_np, "rotT": rotT_np,
            "triT": triT_np, "fmask": fmask_np, "idn": idn_np,
            "ones_col": ones_np, "fvecT": fvecT_np,
        })
    return in_maps


class _Runner:
    """Cached jit + device-resident inputs for repeated NEFF invocations.

    Mirrors concourse.bass2jax.run_bass_via_pjrt's multi-core path, but
    builds the jitted shard_map once (AOT-compiled) and keeps the inputs on
    device, so per-call wall time is launch overhead + device execution.
    """

    def __init__(self, nc, in_maps):
        import jax
        from jax.sharding import Mesh, PartitionSpec, NamedSharding
        from jax.experimental.shard_map import shard_map
        from concourse import bass2jax
        from concourse.bass2jax import _bass_exec_p, install_neuronx_cc_hook

        install_neuronx_cc_hook()
        self.jax = jax
        partition_name = (nc.partition_id_tensor.name
                          if nc.partition_id_tensor else None)

        in_names, out_names, out_avals, zero_outs = [], [], [], []
        for alloc in nc.m.functions[0].allocations:
            if not isinstance(alloc, mybir.MemoryLocationSet):
                continue
            name = alloc.memorylocations[0].name
            if alloc.kind == "ExternalInput":
                if name != partition_name:
                    in_names.append(name)
            elif alloc.kind == "ExternalOutput":
                shape = tuple(alloc.tensor_shape)
                dtype = mybir.dt.np(alloc.dtype)
                out_names.append(name)
                out_avals.append(jax.core.ShapedArray(shape, dtype))
                zero_outs.append(np.zeros(shape, dtype))
        self.in_names, self.out_names = in_names, out_names
        n_params = len(in_names)
        all_in = list(in_names) + list(out_names)
        if partition_name is not None:
            all_in.append(partition_name)

        def _body(*args):
            operands = list(args)
            if partition_name is not None:
                operands.append(bass2jax.partition_id_tensor())
            outs = _bass_exec_p.bind(
                *operands,
                out_avals=tuple(out_avals),
                in_names=tuple(all_in),
                out_names=tuple(out_names),
                lowering_input_output_aliases=(),
                sim_require_finite=True,
                sim_require_nnan=True,
                nc=nc,
            )
            return tuple(outs)

        devices = jax.devices()[:N_CORES]
        mesh = Mesh(np.asarray(devices), ("core",))
        spec = PartitionSpec("core")
        self.sharding = NamedSharding(mesh, spec)
        n_in = n_params + len(zero_outs)
        fn = jax.jit(
            shard_map(_body, mesh=mesh, in_specs=(spec,) * n_in,
                      out_specs=(spec,) * len(out_names), check_rep=False),
            keep_unused=True,
        )
        self.dev_args = []
        for name in in_names:
            conc = np.concatenate([np.asarray(m[name]) for m in in_maps],
                                  axis=0)
            self.dev_args.append(jax.device_put(conc, self.sharding))
        for z in zero_outs:
            conc = np.zeros((N_CORES * z.shape[0], *z.shape[1:]), z.dtype)
            self.dev_args.append(jax.device_put(conc, self.sharding))
        self.fn = fn.lower(*self.dev_args).compile()

    def set_inputs(self, in_maps):
        for i, name in enumerate(self.in_names):
            conc = np.concatenate([np.asarray(m[name]) for m in in_maps],
                                  axis=0)
            self.dev_args[i] = self.jax.device_put(conc, self.sharding)

    def run(self, fetch=False):
        """One blocking execute; returns (wall_seconds, outputs|None)."""
        t0 = time.perf_counter()
        outs = self.fn(*self.dev_args)
        self.jax.block_until_ready(outs)
        t1 = time.perf_counter()
        res = None
        if fetch:
            res = {name: np.asarray(outs[i])
                   for i, name in enumerate(self.out_names)}
        return t1 - t0, res

    def _batch_wall(self, n):
        t0 = time.perf_counter()
        outs = [self.fn(*self.dev_args) for _ in range(n)]
        self.jax.block_until_ready(outs)
        w = time.perf_counter() - t0
        del outs
        return w

    def marginal_ns(self, batches=(2, 14, 26), tries=5):
        """Marginal per-execute wall in pipelined async batches (ns):
        least-squares slope of min-wall over batch size."""
        walls = []
        for n in batches:
            walls.append(min(self._batch_wall(n) for _ in range(tries)))
        xs = np.asarray(batches, np.float64)
        ys = np.asarray(walls, np.float64)
        return float(np.polyfit(xs, ys, 1)[0]) * 1e9


_RUNNERS = {}


def _get_runner(reps, in_maps=None):
    if reps not in _RUNNERS:
        assert in_maps is not None
        _RUNNERS[reps] = _Runner(_build(reps=reps), in_maps)
    elif in_maps is not None:
        _RUNNERS[reps].set_inputs(in_maps)
    return _RUNNERS[reps]


def kernel(hidden_states, cos, sin, Wq, Wk, Wv, Wo):
    in_maps = _host_inputs(hidden_states, cos, sin, Wq, Wk, Wv, Wo)
    runner = _get_runner(1, in_maps)
    _, res = runner.run(fetch=True)
    out = np.empty((S, H), np.float32)
    for j in range(NJ):
        # outQ{j}: concat over cores -> [N_CORES*JW(out cols), JW(seq)]
        qj = np.asarray(res[f"outQ{j}"], np.float32).reshape(N_CORES * JW, JW)
        out[j * JW:(j + 1) * JW, :] = qj.T
    return out.reshape(1, S, H)


def hw_time_ns(reps_hi=8, estimates=3):
    """Slope-based device execution time per kernel invocation (ns).

    Compares marginal per-execute wall time of a NEFF whose body is
    statically unrolled `reps_hi` times against the 1x NEFF; the difference
    divided by (reps_hi - 1) isolates on-device time per invocation,
    including the collectives and their overlap with compute. Marginals are
    interleaved and the median estimate is returned.
    """
    r1 = _RUNNERS.get(1)
    assert r1 is not None, "call kernel() first"
    in_maps = None
    if reps_hi not in _RUNNERS:
        # rebuild per-core in_maps from runner 1's concatenated device args
        in_maps = []
        for c in range(N_CORES):
            m = {}
            for i, name in enumerate(r1.in_names):
                arr = np.asarray(r1.dev_args[i])
                per = arr.shape[0] // N_CORES
                m[name] = arr[c * per:(c + 1) * per]
            in_maps.append(m)
    rh = _get_runner(reps_hi, in_maps)
    vals, m1s, mhs = [], [], []
    for _ in range(estimates):
        m1 = r1.marginal_ns()
        mh = rh.marginal_ns()
        m1s.append(m1)
        mhs.append(mh)
        vals.append((mh - m1) / (reps_hi - 1))
    med = float(np.median(vals))
    return med, float(np.median(m1s)), float(np.median(mhs))


# revision 41
# speedup vs baseline: 1.5654x; 1.5654x over previous
"""Trainium2 Bass kernel for nn_AttnAdapter (GQA attention + RoPE + ClearSight
VAF region scaling + causal softmax), tensor-parallel over heads on 8 cores.

Sharding (Megatron-style TP): core c owns q-heads 4c..4c+3 and kv-head c.
hidden_states^T is replicated to every core (host-side); Wq/Wk/Wv are
column-sharded. o_proj is row-parallel: each core contracts only its own 4
heads over all 4096 output dims, and per seq-quarter a ReduceScatter(add)
sums the partials across cores while scattering output dims, so core c ends
up with its own 512 output columns. The quarter ReduceScatters overlap the
next quarter's attention compute. Output is produced as 4 contiguous
[512, 512] bf16 quarters, assembled + upcast to f32 on the host.

Timing: `_build(reps=R)` statically unrolls the whole body R times (distinct
collective instructions per repetition — collectives inside a hardware loop
desync NRT). HW exec time per invocation = (marginal wall of the R-NEFF -
marginal wall of the 1-NEFF) / (R-1), each marginal measured from pipelined
async execute batches so launch overhead cancels.
"""

import time

import numpy as np
import ml_dtypes

import concourse.bass as bass
import concourse.mybir as mybir
import concourse.tile as tile
from concourse import bacc
from concourse.bass import ts

N_CORES = 8
P = 128
S = 2048
H = 4096
HD = 128
HQ = 4              # q heads per core
JW = 512            # qs super-tile width
NJ = S // JW        # 4
NT = S // P         # 16
KH = H // P         # 32 contraction tiles for projections
SYS, IMG = 35, 576
B = SYS + IMG       # 611: first query row with VAF scaling
ENH, SUP = 2.0, 0.1
FT = -(-B // P)     # 5: ks-tiles with non-unit VAF factor
SCALING = HD ** -0.5

F32 = mybir.dt.float32
MM_DT = mybir.dt.bfloat16
NP_DT = ml_dtypes.bfloat16
RS_DT = MM_DT       # o_proj partials + ReduceScatter in bf16


def _build(reps=1, with_cc=True):
    """with_cc=False drops the ReduceScatters (timing diagnostic only)."""
    nc = bacc.Bacc("TRN2", target_bir_lowering=False, debug=False,
                   num_devices=N_CORES)

    hsT = nc.dram_tensor("hsT", [KH, P, S], MM_DT, kind="ExternalInput")
    wq = nc.dram_tensor("wq", [H, HQ * HD], MM_DT, kind="ExternalInput")
    wk = nc.dram_tensor("wk", [H, HD], MM_DT, kind="ExternalInput")
    wv = nc.dram_tensor("wv", [H, HD], MM_DT, kind="ExternalInput")
    wo = nc.dram_tensor("wo", [HQ * HD, H], MM_DT, kind="ExternalInput")
    cosT = nc.dram_tensor("cosT", [P, S], MM_DT, kind="ExternalInput")
    sinT = nc.dram_tensor("sinT", [P, S], MM_DT, kind="ExternalInput")
    rotT = nc.dram_tensor("rotT", [P, P], MM_DT, kind="ExternalInput")
    triT = nc.dram_tensor("triT", [P, P], MM_DT, kind="ExternalInput")
    fmask = nc.dram_tensor("fmask", [P, FT * P], MM_DT, kind="ExternalInput")
    idn = nc.dram_tensor("idn", [P, P], MM_DT, kind="ExternalInput")
    ones_col = nc.dram_tensor("ones_col", [P, 1], MM_DT, kind="ExternalInput")
    fvecT = nc.dram_tensor("fvecT", [P, FT], F32, kind="ExternalInput")
    outQ = [nc.dram_tensor(f"outQ{j}", [JW, JW], RS_DT, kind="ExternalOutput")
            for j in range(NJ)]

    with tile.TileContext(nc) as tc:
        with (
            tc.tile_pool(name="dram", bufs=1, space="DRAM") as dpool,
            tc.tile_pool(name="consts", bufs=1) as cpool,
            tc.tile_pool(name="qkv", bufs=1) as qkv_pool,
            tc.tile_pool(name="pjw", bufs=1) as pjw,
            tc.tile_pool(name="hs_pool", bufs=8) as hs_pool,
            tc.tile_pool(name="rp_tmp", bufs=4) as rp_tmp,
            tc.tile_pool(name="strip", bufs=4) as strip_pool,
            tc.tile_pool(name="norm", bufs=3) as norm_pool,
            tc.tile_pool(name="fin_pool", bufs=4) as fin_pool,
        ):
            partial = [dpool.tile([H, JW], RS_DT, name=f"partial{j}")
                       for j in range(NJ)]
            outRS = [dpool.tile([JW, JW], RS_DT, name=f"outRS{j}")
                     for j in range(NJ)]

            rot_sb = cpool.tile([P, P], MM_DT, name="rot_sb")
            tri_sb = cpool.tile([P, P], MM_DT, name="tri_sb")
            fm_sb = cpool.tile([P, FT * P], MM_DT, name="fm_sb")
            idn_sb = cpool.tile([P, P], MM_DT, name="idn_sb")
            ones_sb = cpool.tile([P, 1], MM_DT, name="ones_sb")
            fv_sb = cpool.tile([P, FT], F32, name="fv_sb")
            nc.sync.dma_start(rot_sb[:], rotT[:])
            nc.sync.dma_start(tri_sb[:], triT[:])
            nc.sync.dma_start(fm_sb[:], fmask[:])
            nc.sync.dma_start(idn_sb[:], idn[:])
            nc.sync.dma_start(ones_sb[:], ones_col[:])
            nc.sync.dma_start(fv_sb[:], fvecT[:])

            qT = qkv_pool.tile([P, HQ, S], MM_DT, name="qT")
            kT = qkv_pool.tile([P, S], MM_DT, name="kT")
            v_sb = qkv_pool.tile([P, NT, HD], MM_DT, name="v_sb")
            kTs = qkv_pool.tile([P, FT * P], MM_DT, name="kTs")
            oT_sb = qkv_pool.tile([P, HQ, S], MM_DT, name="oT_sb")
            wo_sb = qkv_pool.tile([P, HQ, H], MM_DT, name="wo_sb")
            cos_sb = pjw.tile([P, S], MM_DT, name="cos_sb")
            sin_sb = pjw.tile([P, S], MM_DT, name="sin_sb")
            vT = pjw.tile([P, S], MM_DT, name="vT")
            wq_sb = pjw.tile([P, KH, HQ * HD], MM_DT, name="wq_sb")
            wk_sb = pjw.tile([P, KH, HD], MM_DT, name="wk_sb")
            wv_sb = pjw.tile([P, KH, HD], MM_DT, name="wv_sb")
            wo3 = wo.rearrange("(k p) m -> p k m", p=P)
            wq3 = wq.rearrange("(k p) m -> p k m", p=P)
            wk3 = wk.rearrange("(k p) m -> p k m", p=P)
            wv3 = wv.rearrange("(k p) m -> p k m", p=P)
            WC = 4  # k-tiles per weight-load chunk

            for R in range(reps):
                # ---- Phase 1: projections qT/kT/vT = W^T @ hsT, RoPE,
                #      VAF-scaled kTs, v = transpose(vT) ----
                targets = [qT[:, h, :] for h in range(HQ)] + [kT[:]]
                with (
                    tc.tile_pool(name=f"pj_psum{R}", bufs=1,
                                 space="PSUM") as pj_psum,
                    tc.tile_pool(name=f"rp_psum{R}", bufs=2,
                                 space="PSUM") as rp_psum,
                ):
                  for n in range(NJ):
                    ps_q = [pj_psum.tile([P, JW], F32, tag=f"psq{h}",
                                         name=f"psq{h}_{n}_{R}")
                            for h in range(HQ)]
                    ps_k = pj_psum.tile([P, JW], F32, tag="psk",
                                        name=f"psk_{n}_{R}")
                    ps_v = pj_psum.tile([P, JW], F32, tag="psv",
                                        name=f"psv_{n}_{R}")
                    for k in range(KH):
                        if n == 0 and k % WC == 0:
                            # stream weight chunks just ahead of use
                            nc.sync.dma_start(wq_sb[:, k:k + WC, :],
                                              wq3[:, k:k + WC, :])
                            nc.sync.dma_start(wk_sb[:, k:k + WC, :],
                                              wk3[:, k:k + WC, :])
                            nc.sync.dma_start(wv_sb[:, k:k + WC, :],
                                              wv3[:, k:k + WC, :])
                            # big loads not needed until later: spread them
                            # out so no hst tile queues behind a MB-scale DMA
                            i = k // WC - 1
                            if 0 <= i < HQ:
                                nc.sync.dma_start(wo_sb[:, i, :],
                                                  wo3[:, i, :])
                            elif i == HQ:
                                nc.sync.dma_start(cos_sb[:], cosT[:])
                            elif i == HQ + 1:
                                nc.sync.dma_start(sin_sb[:], sinT[:])
                        hst = hs_pool.tile([P, JW], MM_DT, tag="hs",
                                           name=f"hs_{n}_{k}_{R}")
                        nc.sync.dma_start(hst[:], hsT[k, :, ts(n, JW)])
                        st, sp = (k == 0), (k == KH - 1)
                        for h in range(HQ):
                            nc.tensor.matmul(ps_q[h][:],
                                             wq_sb[:, k, ts(h, HD)],
                                             hst[:], start=st, stop=sp)
                        nc.tensor.matmul(ps_k[:], wk_sb[:, k, :],
                                         hst[:], start=st, stop=sp)
                        nc.tensor.matmul(ps_v[:], wv_sb[:, k, :],
                                         hst[:], start=st, stop=sp)
                    for h in range(HQ):
                        nc.vector.tensor_copy(qT[:, h, ts(n, JW)],
                                              ps_q[h][:])
                    nc.vector.tensor_copy(kT[:, ts(n, JW)], ps_k[:])
                    nc.vector.tensor_copy(vT[:, ts(n, JW)], ps_v[:])
                    # RoPE for this n-slice: x <- x*cos + (Rot@x)*sin
                    for i, tgt in enumerate(targets):
                        rps = rp_psum.tile([P, JW], F32, tag="rp",
                                           name=f"rp_{i}_{n}_{R}")
                        nc.tensor.matmul(rps[:], rot_sb[:],
                                         tgt[:, ts(n, JW)],
                                         start=True, stop=True)
                        tmp = rp_tmp.tile([P, JW], MM_DT, tag="rt",
                                          name=f"rt_{i}_{n}_{R}")
                        nc.vector.tensor_mul(tmp[:], rps[:],
                                             sin_sb[:, ts(n, JW)])
                        nc.vector.tensor_mul(tgt[:, ts(n, JW)],
                                             tgt[:, ts(n, JW)],
                                             cos_sb[:, ts(n, JW)])
                        nc.vector.tensor_add(tgt[:, ts(n, JW)],
                                             tgt[:, ts(n, JW)], tmp[:])

                nc.vector.tensor_mul(kTs[:], kT[:, 0:FT * P], fm_sb[:])

                with tc.tile_pool(name=f"tr_psum{R}", bufs=3,
                                  space="PSUM") as tr_psum:
                    for t in range(NT):
                        tp = tr_psum.tile([P, P], MM_DT, tag="tr",
                                          name=f"tr_{t}_{R}")
                        nc.tensor.transpose(tp[:], vT[:, ts(t, P)],
                                            idn_sb[:])
                        nc.vector.tensor_copy(v_sb[:, t, :], tp[:])

                # ---- Phase 2+3: attention per J-quarter, then row-parallel
                #      o_proj partial and a ReduceScatter that sums the
                #      quarter across cores (overlapping the next quarter) --
                with (
                    tc.tile_pool(name=f"sc_psum{R}", bufs=3,
                                 space="PSUM") as sc_psum,
                    tc.tile_pool(name=f"ao_psum{R}", bufs=2,
                                 space="PSUM") as ao_psum,
                    tc.tile_pool(name=f"dn_psum{R}", bufs=1,
                                 space="PSUM") as dn_psum,
                    tc.tile_pool(name=f"op_psum{R}", bufs=2,
                                 space="PSUM") as op_psum,
                ):
                  for J in range(NJ):
                    qlo, qhi = J * JW, (J + 1) * JW
                    tmax = qhi // P - 1
                    for h in range(HQ):
                        otp = ao_psum.tile([P, JW], F32, tag="ot",
                                           name=f"ot_{h}_{J}_{R}")
                        dnp = dn_psum.tile([1, JW], F32, tag="dn",
                                           name=f"dn_{h}_{J}_{R}")
                        for t in range(tmax + 1):
                            o = max(0, t * P - qlo)
                            scp = sc_psum.tile([P, JW], F32, tag="sc",
                                               name=f"sc_{h}_{J}_{t}_{R}")
                            q_ap = qT[:, h, :]
                            needs_vaf = (t * P < B) and (qhi > B)
                            split = max(o, B - qlo) if needs_vaf else JW
                            if needs_vaf and split == o:
                                # entire strip in the VAF region
                                nc.tensor.matmul(
                                    scp[:, o:JW], kTs[:, ts(t, P)],
                                    q_ap[:, qlo + o:qhi],
                                    start=True, stop=True)
                            else:
                                nc.tensor.matmul(
                                    scp[:, o:JW], kT[:, ts(t, P)],
                                    q_ap[:, qlo + o:qhi],
                                    start=True, stop=True)
                                if needs_vaf and split < JW:
                                    # straddling strip: scale the qs >= B
                                    # columns by the per-ks VAF factor
                                    nc.vector.tensor_scalar_mul(
                                        scp[:, split:JW], scp[:, split:JW],
                                        fv_sb[:, t:t + 1])
                            strip = strip_pool.tile([P, JW], MM_DT, tag="st",
                                                    name=f"st_{h}_{J}_{t}_{R}")
                            nc.scalar.activation(
                                strip[:, o:JW], scp[:, o:JW],
                                mybir.ActivationFunctionType.Exp)
                            if t * P >= qlo:  # diagonal block
                                nc.vector.tensor_mul(strip[:, o:o + P],
                                                     strip[:, o:o + P],
                                                     tri_sb[:])
                            st, sp = (t == 0), (t == tmax)
                            nc.tensor.matmul(otp[:, o:JW],
                                             v_sb[:, t, :],
                                             strip[:, o:JW],
                                             start=st, stop=sp)
                            nc.tensor.matmul(dnp[:, o:JW], ones_sb[:],
                                             strip[:, o:JW],
                                             start=st, stop=sp)
                        recip = norm_pool.tile([1, JW], F32, tag="rc",
                                               name=f"rc_{h}_{J}_{R}")
                        nc.vector.reciprocal(recip[:], dnp[:])
                        bc = norm_pool.tile([P, JW], F32, tag="bc",
                                            name=f"bc_{h}_{J}_{R}")
                        nc.gpsimd.partition_broadcast(bc[:], recip[:])
                        nc.vector.tensor_mul(oT_sb[:, h, ts(J, JW)],
                                             otp[:], bc[:])
                    # row-parallel o_proj partial over own 4 heads only
                    for od in range(KH):
                        pp = op_psum.tile([P, JW], F32, tag="op",
                                          name=f"op_{od}_{J}_{R}")
                        for k in range(HQ):
                            nc.tensor.matmul(
                                pp[:], wo_sb[:, k, ts(od, P)],
                                oT_sb[:, k, ts(J, JW)],
                                start=(k == 0), stop=(k == HQ - 1))
                        fin = fin_pool.tile([P, JW], RS_DT, tag="fin",
                                            name=f"fin_{od}_{J}_{R}")
                        nc.vector.tensor_copy(fin[:], pp[:])
                        nc.sync.dma_start(partial[J][ts(od, P), :], fin[:])
                    # sum partials across cores; core c receives its own
                    # 512 output columns of this seq-quarter, transposed
                    if with_cc:
                        nc.gpsimd.collective_compute(
                            "ReduceScatter", mybir.AluOpType.add,
                            replica_groups=[list(range(N_CORES))],
                            ins=[partial[J].opt()],
                            outs=[outRS[J].opt()],
                        )
                        # collectives cannot write IO tensors directly
                        nc.sync.dma_start(outQ[J][:], outRS[J].opt())

    nc.compile()
    return nc


def _host_inputs(hidden_states, cos, sin, Wq, Wk, Wv, Wo):
    hs2d = np.asarray(hidden_states, dtype=np.float32).reshape(S, H)
    hsT_np = np.ascontiguousarray(hs2d.T).astype(NP_DT).reshape(KH, P, S)
    cosT_np = np.ascontiguousarray(
        np.asarray(cos, np.float32).reshape(S, HD).T).astype(NP_DT)
    sinT_np = np.ascontiguousarray(
        np.asarray(sin, np.float32).reshape(S, HD).T).astype(NP_DT)

    rot = np.zeros((HD, HD), np.float32)
    for i in range(HD // 2):
        rot[i, i + HD // 2] = -1.0
        rot[i + HD // 2, i] = 1.0
    rotT_np = np.ascontiguousarray(rot.T).astype(NP_DT)

    triT_np = np.triu(np.ones((P, P), np.float32)).astype(NP_DT)
    f = np.ones(FT * P, np.float32)
    f[:SYS] = SUP
    f[SYS:B] = ENH
    fmask_np = np.ascontiguousarray(
        np.broadcast_to(f, (P, FT * P))).astype(NP_DT)
    idn_np = np.eye(P, dtype=np.float32).astype(NP_DT)
    ones_np = np.ones((P, 1), np.float32).astype(NP_DT)
    fvecT_np = np.ascontiguousarray(f.reshape(FT, P).T)

    Wq = (np.asarray(Wq, np.float32) * np.float32(SCALING)).astype(NP_DT)
    Wk = np.asarray(Wk, np.float32).astype(NP_DT)
    Wv = np.asarray(Wv, np.float32).astype(NP_DT)
    Wo = np.asarray(Wo, np.float32).astype(NP_DT)

    in_maps = []
    for c in range(N_CORES):
        in_maps.append({
            "hsT": hsT_np,
            "wq": np.ascontiguousarray(Wq[:, c * HQ * HD:(c + 1) * HQ * HD]),
            "wk": np.ascontiguousarray(Wk[:, c * HD:(c + 1) * HD]),
            "wv": np.ascontiguousarray(Wv[:, c * HD:(c + 1) * HD]),
            "wo": np.ascontiguousarray(Wo[c * HQ * HD:(c + 1) * HQ * HD, :]),
            "cosT": cosT_np, "sinT": sinT_np, "rotT": rotT_np,
            "triT": triT_np, "fmask": fmask_np, "idn": idn_np,
            "ones_col": ones_np, "fvecT": fvecT_np,
        })
    return in_maps


class _Runner:
    """Cached jit + device-resident inputs for repeated NEFF invocations.

    Mirrors concourse.bass2jax.run_bass_via_pjrt's multi-core path, but
    builds the jitted shard_map once (AOT-compiled) and keeps the inputs on
    device, so per-call wall time is launch overhead + device execution.
    """

    def __init__(self, nc, in_maps):
        import jax
        from jax.sharding import Mesh, PartitionSpec, NamedSharding
        from jax.experimental.shard_map import shard_map
        from concourse import bass2jax
        from concourse.bass2jax import _bass_exec_p, install_neuronx_cc_hook

        install_neuronx_cc_hook()
        self.jax = jax
        partition_name = (nc.partition_id_tensor.name
                          if nc.partition_id_tensor else None)

        in_names, out_names, out_avals, zero_outs = [], [], [], []
        for alloc in nc.m.functions[0].allocations:
            if not isinstance(alloc, mybir.MemoryLocationSet):
                continue
            name = alloc.memorylocations[0].name
            if alloc.kind == "ExternalInput":
                if name != partition_name:
                    in_names.append(name)
            elif alloc.kind == "ExternalOutput":
                shape = tuple(alloc.tensor_shape)
                dtype = mybir.dt.np(alloc.dtype)
                out_names.append(name)
                out_avals.append(jax.core.ShapedArray(shape, dtype))
                zero_outs.append(np.zeros(shape, dtype))
        self.in_names, self.out_names = in_names, out_names
        n_params = len(in_names)
        all_in = list(in_names) + list(out_names)
        if partition_name is not None:
            all_in.append(partition_name)

        def _body(*args):
            operands = list(args)
            if partition_name is not None:
                operands.append(bass2jax.partition_id_tensor())
            outs = _bass_exec_p.bind(
                *operands,
                out_avals=tuple(out_avals),
                in_names=tuple(all_in),
                out_names=tuple(out_names),
                lowering_input_output_aliases=(),
                sim_require_finite=True,
                sim_require_nnan=True,
                nc=nc,
            )
            return tuple(outs)

        devices = jax.devices()[:N_CORES]
        mesh = Mesh(np.asarray(devices), ("core",))
        spec = PartitionSpec("core")
        self.sharding = NamedSharding(mesh, spec)
        n_in = n_params + len(zero_outs)
        fn = jax.jit(
            shard_map(_body, mesh=mesh, in_specs=(spec,) * n_in,
                      out_specs=(spec,) * len(out_names), check_rep=False),
            keep_unused=True,
        )
        self.dev_args = []
        for name in in_names:
            conc = np.concatenate([np.asarray(m[name]) for m in in_maps],
                                  axis=0)
            self.dev_args.append(jax.device_put(conc, self.sharding))
        for z in zero_outs:
            conc = np.zeros((N_CORES * z.shape[0], *z.shape[1:]), z.dtype)
            self.dev_args.append(jax.device_put(conc, self.sharding))
        self.fn = fn.lower(*self.dev_args).compile()

    def set_inputs(self, in_maps):
        for i, name in enumerate(self.in_names):
            conc = np.concatenate([np.asarray(m[name]) for m in in_maps],
                                  axis=0)
            self.dev_args[i] = self.jax.device_put(conc, self.sharding)

    def run(self, fetch=False):
        """One blocking execute; returns (wall_seconds, outputs|None)."""
        t0 = time.perf_counter()
        outs = self.fn(*self.dev_args)
        self.jax.block_until_ready(outs)
        t1 = time.perf_counter()
        res = None
        if fetch:
            res = {name: np.asarray(outs[i])
                   for i, name in enumerate(self.out_names)}
        return t1 - t0, res

    def _batch_wall(self, n):
        t0 = time.perf_counter()
        outs = [self.fn(*self.dev_args) for _ in range(n)]
        self.jax.block_until_ready(outs)
        w = time.perf_counter() - t0
        del outs
        return w

    def marginal_ns(self, batches=(2, 14, 26), tries=5):
        """Marginal per-execute wall in pipelined async batches (ns):
        least-squares slope of min-wall over batch size."""
        walls = []
        for n in batches:
            walls.append(min(self._batch_wall(n) for _ in range(tries)))
        xs = np.asarray(batches, np.float64)
        ys = np.asarray(walls, np.float64)
        return float(np.polyfit(xs, ys, 1)[0]) * 1e9


_RUNNERS = {}


def _get_runner(reps, in_maps=None):
    if reps not in _RUNNERS:
        assert in_maps is not None
        _RUNNERS[reps] = _Runner(_build(reps=reps), in_maps)
    elif in_maps is not None:
        _RUNNERS[reps].set_inputs(in_maps)
    return _RUNNERS[reps]


def kernel(hidden_states, cos, sin, Wq, Wk, Wv, Wo):
    in_maps = _host_inputs(hidden_states, cos, sin, Wq, Wk, Wv, Wo)
    runner = _get_runner(1, in_maps)
    _, res = runner.run(fetch=True)
    out = np.empty((S, H), np.float32)
    for j in range(NJ):
        # outQ{j}: concat over cores -> [N_CORES*JW(out cols), JW(seq)]
        qj = np.asarray(res[f"outQ{j}"], np.float32).reshape(N_CORES * JW, JW)
        out[j * JW:(j + 1) * JW, :] = qj.T
    return out.reshape(1, S, H)


def hw_time_ns(reps_hi=8, estimates=3):
    """Slope-based device execution time per kernel invocation (ns).

    Compares marginal per-execute wall time of a NEFF whose body is
    statically unrolled `reps_hi` times against the 1x NEFF; the difference
    divided by (reps_hi - 1) isolates on-device time per invocation,
    including the collectives and their overlap with compute. Marginals are
    interleaved and the median estimate is returned.
    """
    r1 = _RUNNERS.get(1)
    assert r1 is not None, "call kernel() first"
    in_maps = None
    if reps_hi not in _RUNNERS:
        # rebuild per-core in_maps from runner 1's concatenated device args
        in_maps = []
        for c in range(N_CORES):
            m = {}
            for i, name in enumerate(r1.in_names):
                arr = np.asarray(r1.dev_args[i])
                per = arr.shape[0] // N_CORES
                m[name] = arr[c * per:(c + 1) * per]
            in_maps.append(m)
    rh = _get_runner(reps_hi, in_maps)
    vals, m1s, mhs = [], [], []
    for _ in range(estimates):
        m1 = r1.marginal_ns()
        mh = rh.marginal_ns()
        m1s.append(m1)
        mhs.append(mh)
        vals.append((mh - m1) / (reps_hi - 1))
    med = float(np.median(vals))
    return med, float(np.median(m1s)), float(np.median(mhs))
